# revision 28
# baseline (speedup 1.0000x reference)
"""Trainium2 Bass kernel for ArccosHessianCalculator (triplet arccos-Hessian
weight-diagonal).

Math (per pair (x1, x2), z = x @ W.T):
  s1 = ||z1||^2, s2 = ||z2||^2, s12 = z1.z2  (rowwise)
  r1 = 1/s1, r2 = 1/s2, g = sqrt(r1*r2), c = s12*g
  Only the DIAGONALS of the b x d x d Hessians are needed:
    d11 = 2(g r1) P - 3(c r1^2) Q1 + c r1
    -2*d12 = 2(c g^2) P - 2(g r1) Q1 - 2(g r2) Q2 + 2g
    d22 = 2(g r2) P - 3(c r2^2) Q2 + c r2
  with P = z1*z2, Q1 = z1^2, Q2 = z2^2 (elementwise [b, d_out]).
  out[o, j] = sum_b d11*x1[j]^2 + (-2 d12)*x1[j]x2[j] + d22*x2[j]^2
  result = pos_pair - neg_pair  (sign folded into g for the neg pair).

Distribution: data-parallel over the tuple dim b (1024 = 8 cores x 128).
Each core gathers its 4x128 rows of x (cast to fp16 in the DMA), computes a
partial [256, 512] fp16 weight-diagonal, then a ReduceScatter(add) leaves
rows [32k:32k+32] of the global sum on core k; the host concatenates the
8 shards.

All TensorEngine operands are fp16 (same PE throughput as bf16, 8x the
mantissa precision); accumulations (PSUM, activation accumulators, row
scalars) stay fp32.  The exec time is dominated by the runtime's collective
path: a runtime-inserted entry barrier completes only at ~57-66us (CC
firmware boot ~21-33us per core + launch skew + ~16us protocol), the first
user collective starts ~11us after that, the RS itself takes ~11-15us.
Local compute (~26us) is entirely hidden under that fixed latency.
"""

import os
import sys

import numpy as np

for _p in ("/opt/trn_rl_repo", "/root/.axon_site/_ro/trn_rl_repo"):
    if os.path.isdir(_p) and _p not in sys.path:
        sys.path.append(_p)

from concourse import bacc, bass, mybir, tile
from concourse.bass_utils import run_bass_kernel_spmd
from concourse.instruction_name_ordered_set import InstructionNameOrderedSet


def _dep_set(names):
    s = InstructionNameOrderedSet()
    for n in names:
        s.add(n)
    return s

N_CORES = 8
N_ROWS, D_IN, D_OUT, B = 16384, 512, 256, 1024
BL = B // N_CORES          # 128 tuples per core
KC = D_IN // 128           # 4 contraction chunks
OC = D_OUT // 128          # 2 output-row chunks
OUT_SH = D_OUT // N_CORES  # 32 rows per core after ReduceScatter

F32 = mybir.dt.float32
F16 = mybir.dt.float16
ALU = mybir.AluOpType
ACT_F = mybir.ActivationFunctionType

PROFILE = False
LAST_EXEC_NS = None
LAST_RESULTS = None

FUSED_GATHER = False
XCHG_PREPS = True
XCHG_TRIGGER = True
XCHG_WAIT = False

_CACHED_NC = None


def _build():
    nc = bacc.Bacc(
        "TRN2",
        target_bir_lowering=False,
        debug=False,
        num_devices=N_CORES,
    )

    x_d = nc.dram_tensor("xfull", [N_ROWS, D_IN], F32, kind="ExternalInput")
    wt_d = nc.dram_tensor("wt", [128, KC * D_OUT], F32, kind="ExternalInput")
    idx_d = nc.dram_tensor("idx", [128, 4], mybir.dt.int32, kind="ExternalInput")
    ident_d = nc.dram_tensor("ident", [128, 128], F16, kind="ExternalInput")
    out_d = nc.dram_tensor("out", [OUT_SH, D_IN], F16, kind="ExternalOutput")

    with tile.TileContext(nc) as tc:
        with (
            tc.tile_pool(name="const", bufs=1) as constp,
            tc.tile_pool(name="xg", bufs=1) as xgp,
            tc.tile_pool(name="xt", bufs=4) as xtp,
            tc.tile_pool(name="pq", bufs=2) as pqp,
            tc.tile_pool(name="dd", bufs=2) as ddp,
            tc.tile_pool(name="xx", bufs=2) as xxp,
            tc.tile_pool(name="sc", bufs=2) as scp,
            tc.tile_pool(name="osb", bufs=1) as osbp,
            tc.tile_pool(name="pt", bufs=2, space="PSUM") as ptp,
            tc.tile_pool(name="pz", bufs=4, space="PSUM") as pzp,
            tc.tile_pool(name="po", bufs=2, space="PSUM") as pop,
            tc.tile_pool(name="dram", bufs=1, space="DRAM") as dramp,
        ):
            idx_sb = constp.tile([128, 4], mybir.dt.int32, tag="idx")
            wt_sb = constp.tile([128, KC, D_OUT], F32, tag="wt")
            ident_sb = constp.tile([128, 128], F16, tag="ident")

            nc.sync.dma_start(idx_sb[:], idx_d[:])
            nc.sync.dma_start(ident_sb[:], ident_d[:])
            nc.sync.dma_start(
                wt_sb[:], wt_d.ap().rearrange("p (c o) -> p c o", c=KC)
            )
            wtb = constp.tile([128, KC, D_OUT], F16, tag="wtb")
            nc.scalar.activation(wtb[:], wt_sb[:], ACT_F.Copy)

            # force ACT table loads (Square/Sqrt/Identity) off the critical path
            dum_in = scp.tile([128, 1], F32, tag="dum_in")
            dum_o = scp.tile([128, 3], F32, tag="dum_o")
            nc.vector.memset(dum_in[:], 1.0)
            nc.scalar.activation(dum_o[:, 0:1], dum_in[:], ACT_F.Square)
            nc.scalar.activation(dum_o[:, 1:2], dum_in[:], ACT_F.Sqrt)
            nc.scalar.activation(dum_o[:, 2:3], dum_in[:], ACT_F.Identity)

            # --- gather the 4 x-tensors as fp16: xgb[:, t, :] = x[idx_t] ---
            xgb = xgp.tile([128, 4, D_IN], F16, tag="xgb")
            if FUSED_GATHER:
                nc.gpsimd.indirect_dma_start(
                    out=xgb[:, :, :],
                    out_offset=None,
                    in_=x_d[:],
                    in_offset=bass.IndirectOffsetOnAxis(
                        ap=idx_sb[:, 0:4], axis=0
                    ),
                )
            else:
                for t in range(4):
                    nc.gpsimd.indirect_dma_start(
                        out=xgb[:, t, :],
                        out_offset=None,
                        in_=x_d[:],
                        in_offset=bass.IndirectOffsetOnAxis(
                            ap=idx_sb[:, t : t + 1], axis=0
                        ),
                    )

            osb = osbp.tile([128, OC, D_IN], F16, tag="osb")

            # --- xx products (fp16, off the z critical path) ---
            xx_all = []
            for pi, (i, j) in enumerate([(0, 1), (2, 3)]):
                xx1 = xxp.tile([128, D_IN], F16, tag="xx1", name=f"xx1_{pi}")
                x12 = xxp.tile([128, D_IN], F16, tag="x12", name=f"x12_{pi}")
                xx2 = xxp.tile([128, D_IN], F16, tag="xx2", name=f"xx2_{pi}")
                xi = xgb[:, i, :]
                xj = xgb[:, j, :]
                nc.gpsimd.tensor_tensor(xx1[:], xi, xi, ALU.mult)
                nc.vector.tensor_tensor(x12[:], xi, xj, ALU.mult)
                nc.gpsimd.tensor_tensor(xx2[:], xj, xj, ALU.mult)
                xx_all.append((xx1, x12, xx2))

            # --- transpose each gathered tensor: xt[t] [128(k), c, 128(b)] ---
            xts = []
            for t in range(4):
                xt = xtp.tile([128, KC, 128], F16, tag="xt", name=f"xt{t}")
                for c in range(KC):
                    pt = ptp.tile([128, 128], F16, tag="pt")
                    nc.tensor.transpose(
                        pt[:],
                        xgb[:, t, c * 128 : (c + 1) * 128],
                        ident_sb[:],
                    )
                    if (t * KC + c) % 2 == 0:
                        nc.vector.tensor_copy(xt[:, c, :], pt[:])
                    else:
                        nc.scalar.copy(xt[:, c, :], pt[:])
                xts.append(xt)

            # --- z matmuls: zps[t] [128(b), 256(o)] f32 in PSUM ---
            zps = []
            for t in range(4):
                zp = pzp.tile([128, D_OUT], F32, tag="z", name=f"z{t}")
                for c in range(KC):
                    nc.tensor.matmul(
                        zp[:],
                        xts[t][:, c, :],
                        wtb[:, c, :],
                        start=(c == 0),
                        stop=(c == KC - 1),
                    )
                zps.append(zp)

            vtt = nc.vector.tensor_tensor
            vts = nc.vector.tensor_scalar

            # --- row scalars ---
            # s4 cols: [s_i_pos, s_i_neg, s_j_pos, s_j_neg]
            s4 = scp.tile([128, 4], F32, tag="s4")
            q_all = []
            for t in range(4):
                qt = pqp.tile([128, D_OUT], F16, tag="q", name=f"q{t}", bufs=4)
                col = (t % 2) * 2 + (t // 2)
                nc.scalar.activation(
                    qt[:], zps[t][:], ACT_F.Square, accum_out=s4[:, col : col + 1]
                )
                q_all.append(qt)

            def pk(tag, w=2):
                return scp.tile([128, w], F32, tag=f"pk_{tag}", name=f"pk_{tag}")

            # P products + s12 accumulation
            s12_2 = pk("s12_2")
            pp_l = []
            for pi, (i, j) in enumerate([(0, 1), (2, 3)]):
                zbj = pqp.tile([128, D_OUT], F16, tag="zb", name=f"zb{pi}")
                nc.vector.tensor_copy(zbj[:], zps[j][:])
                pp = pqp.tile([128, D_OUT], F16, tag="pp", name=f"pp_{pi}")
                vtt(pp[:], zbj[:], zps[i][:], ALU.mult)
                junk = pqp.tile([128, D_OUT], F16, tag="junk", name=f"junk_{pi}")
                nc.scalar.activation(
                    junk[:], pp[:], ACT_F.Identity,
                    accum_out=s12_2[:, pi : pi + 1],
                )
                pp_l.append((pp, junk))

            # reciprocals + g + c  (r4 cols mirror s4)
            r4 = pk("r4", 4)
            nc.vector.reciprocal(r4[:], s4[:])
            ri2, rj2 = r4[:, 0:2], r4[:, 2:4]
            rr2 = pk("rr2")
            vtt(rr2[:], ri2, rj2, ALU.mult)
            g2 = pk("g2")
            nc.scalar.activation(g2[:], rr2[:], ACT_F.Sqrt)
            # fold the neg-pair sign into g (all coefficients are odd in g)
            vts(g2[:, 1:2], g2[:, 1:2], -1.0, None, ALU.mult)
            c2_ = pk("c2_")
            vtt(c2_[:], s12_2[:], g2[:], ALU.mult)

            gri2, grj2, cri2, crj2, cg2 = (
                pk("gri2"), pk("grj2"), pk("cri2"), pk("crj2"), pk("cg2")
            )
            vtt(gri2[:], g2[:], ri2, ALU.mult)
            vtt(grj2[:], g2[:], rj2, ALU.mult)
            vtt(cri2[:], c2_[:], ri2, ALU.mult)
            vtt(crj2[:], c2_[:], rj2, ALU.mult)
            vtt(cg2[:], c2_[:], g2[:], ALU.mult)
            a11_2, a22_2, e12_2 = pk("a11_2"), pk("a22_2"), pk("e12_2")
            m11_2, m22_2, a12_2 = pk("m11_2"), pk("m22_2"), pk("a12_2")
            m12i_2, m12j_2 = pk("m12i_2"), pk("m12j_2")
            mt1, mt2, at = pk("mt1"), pk("mt2"), pk("at")
            vts(a11_2[:], gri2[:], 2.0, None, ALU.mult)
            vtt(mt1[:], cri2[:], ri2, ALU.mult)
            vts(m11_2[:], mt1[:], -3.0, None, ALU.mult)
            vts(a22_2[:], grj2[:], 2.0, None, ALU.mult)
            vtt(mt2[:], crj2[:], rj2, ALU.mult)
            vts(m22_2[:], mt2[:], -3.0, None, ALU.mult)
            vtt(at[:], cg2[:], g2[:], ALU.mult)
            vts(a12_2[:], at[:], 2.0, None, ALU.mult)
            vts(m12i_2[:], gri2[:], -2.0, None, ALU.mult)
            vts(m12j_2[:], grj2[:], -2.0, None, ALU.mult)
            vts(e12_2[:], g2[:], 2.0, None, ALU.mult)

            # --- per-pair D assembly (fp16 outputs) ---
            d_all = []
            for pi, (i, j) in enumerate([(0, 1), (2, 3)]):
                q1, q2 = q_all[i], q_all[j]
                pp = pp_l[pi][0]
                sl = slice(pi, pi + 1)
                d11 = ddp.tile([128, D_OUT], F16, tag="d11", name=f"d11_{pi}")
                d12 = ddp.tile([128, D_OUT], F16, tag="d12", name=f"d12_{pi}")
                d22 = ddp.tile([128, D_OUT], F16, tag="d22", name=f"d22_{pi}")
                t2 = pqp.tile([128, D_OUT], F16, tag="t2", name=f"t2_{pi}")
                nc.scalar.activation(
                    t2[:], q1[:], ACT_F.Identity,
                    bias=cri2[:, sl], scale=m11_2[:, sl],
                )
                nc.vector.scalar_tensor_tensor(
                    d11[:], pp[:], a11_2[:, sl], t2[:], ALU.mult, ALU.add
                )
                t4 = pqp.tile([128, D_OUT], F16, tag="t4", name=f"t4_{pi}")
                nc.scalar.activation(
                    t4[:], q2[:], ACT_F.Identity,
                    bias=crj2[:, sl], scale=m22_2[:, sl],
                )
                nc.vector.scalar_tensor_tensor(
                    d22[:], pp[:], a22_2[:, sl], t4[:], ALU.mult, ALU.add
                )
                t6 = pqp.tile([128, D_OUT], F16, tag="t6", name=f"t6_{pi}")
                nc.scalar.activation(
                    t6[:], q1[:], ACT_F.Identity,
                    bias=e12_2[:, sl], scale=m12i_2[:, sl],
                )
                u1 = pqp.tile([128, D_OUT], F16, tag="u1", name=f"u1_{pi}")
                nc.vector.scalar_tensor_tensor(
                    u1[:], pp[:], a12_2[:, sl], t6[:], ALU.mult, ALU.add
                )
                nc.vector.scalar_tensor_tensor(
                    d12[:], q2[:], m12j_2[:, sl], u1[:], ALU.mult, ALU.add
                )
                d_all.append((d11, d12, d22))

            # --- keep-warm transposes: PE idles while d's assemble; stop the
            # HAM clock gate from re-throttling before the final matmuls ---
            for wi, warm_src in enumerate(
                (q_all[0][:, 0:128], pp_l[0][1][:, 0:128], pp_l[1][1][:, 0:128])
            ):
                ptd = ptp.tile([128, 128], F16, tag="pt", name=f"ptd{wi}")
                nc.tensor.transpose(ptd[:], warm_src, ident_sb[:])

            # --- final accumulation matmuls: out[o, j] in PSUM [128, 512] ---
            terms = []
            for pi in range(2):
                for k in range(3):
                    terms.append((d_all[pi][k], xx_all[pi][k]))
            pouts = [
                pop.tile([128, D_IN], F32, tag="pout", name=f"pout{mc}")
                for mc in range(OC)
            ]
            for k, (dmat, xmat) in enumerate(terms):
                for mc in range(OC):
                    nc.tensor.matmul(
                        pouts[mc][:],
                        dmat[:, mc * 128 : (mc + 1) * 128],
                        xmat[:],
                        start=(k == 0),
                        stop=(k == len(terms) - 1),
                    )
            for mc in range(OC):
                nc.vector.tensor_copy(osb[:, mc, :], pouts[mc][:])

            # --- ReduceScatter across the 8 cores, then write the shard ---
            rs_in = dramp.tile([D_OUT, D_IN], F16, tag="rs_in")
            rs_out = dramp.tile([OUT_SH, D_IN], F16, tag="rs_out")
            nc.sync.dma_start(
                rs_in[:].rearrange("(c p) o -> p c o", p=128), osb[:]
            )
            nc.gpsimd.collective_compute(
                "ReduceScatter",
                ALU.add,
                replica_groups=[list(range(N_CORES))],
                ins=[rs_in[:].opt()],
                outs=[rs_out[:].opt()],
            )
            nc.sync.dma_start(out_d[:], rs_out[:])

    nc.compile()
    return nc


def _get_nc():
    global _CACHED_NC
    if _CACHED_NC is None:
        _CACHED_NC = _build()
    return _CACHED_NC


def _pack_inputs(x, W, ap, p, an, n):
    x = np.ascontiguousarray(np.asarray(x, dtype=np.float32))
    W = np.asarray(W, dtype=np.float32)
    wt_packed = np.ascontiguousarray(
        W.T.reshape(KC, 128, D_OUT).transpose(1, 0, 2)
    ).reshape(128, KC * D_OUT)
    ident = np.eye(128, dtype=np.float16)
    idxs = [np.asarray(a).astype(np.int64) for a in (ap, p, an, n)]
    in_maps = []
    for core in range(N_CORES):
        sl = slice(core * BL, (core + 1) * BL)
        idx_core = np.ascontiguousarray(
            np.stack([a[sl] for a in idxs], axis=1).astype(np.int32)
        )  # [128, 4]
        in_maps.append(
            {"xfull": x, "wt": wt_packed, "idx": idx_core, "ident": ident}
        )
    return in_maps


def kernel(x, W, ap, p, an, n):
    global LAST_EXEC_NS, LAST_RESULTS
    nc = _get_nc()
    in_maps = _pack_inputs(x, W, ap, p, an, n)
    kw = {}
    if PROFILE:
        kw = dict(trace=True)
    res = run_bass_kernel_spmd(nc, in_maps, list(range(N_CORES)), **kw)
    LAST_EXEC_NS = res.exec_time_ns
    LAST_RESULTS = res
    shards = [
        np.asarray(res.results[i]["out"]).astype(np.float32)
        for i in range(N_CORES)
    ]
    full = np.concatenate(shards, axis=0).reshape(-1)
    return np.ascontiguousarray(full)


# revision 31
# speedup vs baseline: 1.2017x; 1.2017x over previous
"""Trainium2 Bass kernel for ArccosHessianCalculator (triplet arccos-Hessian
weight-diagonal).

Math (per pair (x1, x2), z = x @ W.T):
  s1 = ||z1||^2, s2 = ||z2||^2, s12 = z1.z2  (rowwise)
  r1 = 1/s1, r2 = 1/s2, g = sqrt(r1*r2), c = s12*g
  Only the DIAGONALS of the b x d x d Hessians are needed:
    d11 = 2(g r1) P - 3(c r1^2) Q1 + c r1
    -2*d12 = 2(c g^2) P - 2(g r1) Q1 - 2(g r2) Q2 + 2g
    d22 = 2(g r2) P - 3(c r2^2) Q2 + c r2
  with P = z1*z2, Q1 = z1^2, Q2 = z2^2 (elementwise [b, d_out]).
  out[o, j] = sum_b d11*x1[j]^2 + (-2 d12)*x1[j]x2[j] + d22*x2[j]^2
  result = pos_pair - neg_pair  (sign folded into g for the neg pair).

Distribution: data-parallel over the tuple dim b (1024 = 8 cores x 128).
Each core gathers its 4x128 rows of x (cast to fp16 in the DMA), computes a
partial [256, 512] fp16 weight-diagonal, then a ReduceScatter(add) leaves
rows [32k:32k+32] of the global sum on core k; the host concatenates the
8 shards.

All TensorEngine operands are fp16 (same PE throughput as bf16, 8x the
mantissa precision); accumulations (PSUM, activation accumulators, row
scalars) stay fp32.  The exec time is dominated by the runtime's collective
path: a runtime-inserted entry barrier completes only at ~57-66us (CC
firmware boot ~21-33us per core + launch skew + ~16us protocol), the first
user collective starts ~11us after that, the RS itself takes ~11-15us.
Local compute (~26us) is entirely hidden under that fixed latency.
"""

import os
import sys

import numpy as np

for _p in ("/opt/trn_rl_repo", "/root/.axon_site/_ro/trn_rl_repo"):
    if os.path.isdir(_p) and _p not in sys.path:
        sys.path.append(_p)

from concourse import bacc, bass, mybir, tile
from concourse.bass_utils import run_bass_kernel_spmd
from concourse.instruction_name_ordered_set import InstructionNameOrderedSet


def _dep_set(names):
    s = InstructionNameOrderedSet()
    for n in names:
        s.add(n)
    return s

N_CORES = 8
N_ROWS, D_IN, D_OUT, B = 16384, 512, 256, 1024
BL = B // N_CORES          # 128 tuples per core
KC = D_IN // 128           # 4 contraction chunks
OC = D_OUT // 128          # 2 output-row chunks
OUT_SH = D_OUT // N_CORES  # 32 rows per core after ReduceScatter
OG = 2                     # output groups (128 out-rows each)
BG = N_CORES // OG         # 4 batch groups per output group
NBLK = B // BG // 128      # 2 tuple-blocks of 128 per core
OSL = D_OUT // OG          # 128 out-rows owned per core (first cols after perm)

F32 = mybir.dt.float32
F16 = mybir.dt.float16
ALU = mybir.AluOpType
ACT_F = mybir.ActivationFunctionType

PROFILE = False
LAST_EXEC_NS = None
LAST_RESULTS = None

FUSED_GATHER = False
XCHG_PREPS = True
XCHG_TRIGGER = True
XCHG_WAIT = False

_CACHED_NC = None


def _build():
    nc = bacc.Bacc(
        "TRN2",
        target_bir_lowering=False,
        debug=False,
        num_devices=N_CORES,
    )

    x_d = nc.dram_tensor("xfull", [N_ROWS, D_IN], F32, kind="ExternalInput")
    wt_d = nc.dram_tensor("wt", [128, KC * D_OUT], F32, kind="ExternalInput")
    idx_d = nc.dram_tensor("idx", [128, 4 * NBLK], mybir.dt.int32, kind="ExternalInput")
    ident_d = nc.dram_tensor("ident", [128, 128], F16, kind="ExternalInput")
    out_d = nc.dram_tensor("out", [OUT_SH, D_IN], F16, kind="ExternalOutput")

    with tile.TileContext(nc) as tc:
        with (
            tc.tile_pool(name="const", bufs=1) as constp,
            tc.tile_pool(name="xg", bufs=1) as xgp,
            tc.tile_pool(name="xt", bufs=4) as xtp,
            tc.tile_pool(name="pq", bufs=2) as pqp,
            tc.tile_pool(name="dd", bufs=2) as ddp,
            tc.tile_pool(name="xx", bufs=2) as xxp,
            tc.tile_pool(name="sc", bufs=2) as scp,
            tc.tile_pool(name="osb", bufs=1) as osbp,
            tc.tile_pool(name="pt", bufs=2, space="PSUM") as ptp,
            tc.tile_pool(name="pz", bufs=4, space="PSUM") as pzp,
            tc.tile_pool(name="po", bufs=2, space="PSUM") as pop,
            tc.tile_pool(name="dram", bufs=1, space="DRAM") as dramp,
        ):
            idx_sb = constp.tile([128, 4 * NBLK], mybir.dt.int32, tag="idx")
            wt_sb = constp.tile([128, KC, D_OUT], F32, tag="wt")
            ident_sb = constp.tile([128, 128], F16, tag="ident")

            nc.sync.dma_start(idx_sb[:], idx_d[:])
            nc.sync.dma_start(ident_sb[:], ident_d[:])
            nc.sync.dma_start(
                wt_sb[:], wt_d.ap().rearrange("p (c o) -> p c o", c=KC)
            )
            wtb = constp.tile([128, KC, D_OUT], F16, tag="wtb")
            nc.scalar.activation(wtb[:], wt_sb[:], ACT_F.Copy)

            # force ACT table loads (Square/Sqrt/Identity) off the critical path
            dum_in = scp.tile([128, 1], F32, tag="dum_in")
            dum_o = scp.tile([128, 3], F32, tag="dum_o")
            nc.vector.memset(dum_in[:], 1.0)
            nc.scalar.activation(dum_o[:, 0:1], dum_in[:], ACT_F.Square)
            nc.scalar.activation(dum_o[:, 1:2], dum_in[:], ACT_F.Sqrt)
            nc.scalar.activation(dum_o[:, 2:3], dum_in[:], ACT_F.Identity)

            # --- gather 2 blocks x 4 x-tensors as fp16 ---
            xgb = xgp.tile([128, 4 * NBLK, D_IN], F16, tag="xgb")
            for t in range(4 * NBLK):
                nc.gpsimd.indirect_dma_start(
                    out=xgb[:, t, :],
                    out_offset=None,
                    in_=x_d[:],
                    in_offset=bass.IndirectOffsetOnAxis(
                        ap=idx_sb[:, t : t + 1], axis=0
                    ),
                )

            osb = osbp.tile([128, D_IN], F16, tag="osb")

            pout = pop.tile([128, D_IN], F32, tag="pout")
            vtt = nc.vector.tensor_tensor
            vts = nc.vector.tensor_scalar

            for blk in range(NBLK):
                tb = [blk * 4 + t for t in range(4)]

                # --- xx products (fp16, off the z critical path) ---
                xx_all = []
                for pi, (i, j) in enumerate([(0, 1), (2, 3)]):
                    xx1 = xxp.tile([128, D_IN], F16, tag="xx1", name=f"xx1_{blk}_{pi}")
                    x12 = xxp.tile([128, D_IN], F16, tag="x12", name=f"x12_{blk}_{pi}")
                    xx2 = xxp.tile([128, D_IN], F16, tag="xx2", name=f"xx2_{blk}_{pi}")
                    xi = xgb[:, tb[i], :]
                    xj = xgb[:, tb[j], :]
                    nc.gpsimd.tensor_tensor(xx1[:], xi, xi, ALU.mult)
                    nc.vector.tensor_tensor(x12[:], xi, xj, ALU.mult)
                    nc.gpsimd.tensor_tensor(xx2[:], xj, xj, ALU.mult)
                    xx_all.append((xx1, x12, xx2))

                # --- transpose each gathered tensor: xt [128(k), c, 128(b)] ---
                xts = []
                for t in range(4):
                    xt = xtp.tile([128, KC, 128], F16, tag="xt", name=f"xt{blk}_{t}")
                    for c in range(KC):
                        pt = ptp.tile([128, 128], F16, tag="pt")
                        nc.tensor.transpose(
                            pt[:],
                            xgb[:, tb[t], c * 128 : (c + 1) * 128],
                            ident_sb[:],
                        )
                        if (t * KC + c) % 2 == 0:
                            nc.vector.tensor_copy(xt[:, c, :], pt[:])
                        else:
                            nc.scalar.copy(xt[:, c, :], pt[:])
                    xts.append(xt)

                # --- z matmuls: zps[t] [128(b), 256(o-permuted)] f32 ---
                zps = []
                for t in range(4):
                    zp = pzp.tile([128, D_OUT], F32, tag="z", name=f"z{blk}_{t}")
                    for c in range(KC):
                        nc.tensor.matmul(
                            zp[:],
                            xts[t][:, c, :],
                            wtb[:, c, :],
                            start=(c == 0),
                            stop=(c == KC - 1),
                        )
                    zps.append(zp)

                # --- row scalars (accumulated over the FULL 256-o width) ---
                s4 = scp.tile([128, 4], F32, tag="s4", name=f"s4_{blk}")
                q_all = []
                for t in range(4):
                    qt = pqp.tile([128, D_OUT], F16, tag="q", name=f"q{blk}_{t}", bufs=4)
                    col = (t % 2) * 2 + (t // 2)
                    nc.scalar.activation(
                        qt[:], zps[t][:], ACT_F.Square,
                        accum_out=s4[:, col : col + 1],
                    )
                    q_all.append(qt)

                def pk(tag, w=2):
                    return scp.tile(
                        [128, w], F32, tag=f"pk_{tag}", name=f"pk_{blk}_{tag}"
                    )

                s12_2 = pk("s12_2")
                pp_l = []
                for pi, (i, j) in enumerate([(0, 1), (2, 3)]):
                    zbj = pqp.tile([128, D_OUT], F16, tag="zb", name=f"zb{blk}_{pi}")
                    nc.vector.tensor_copy(zbj[:], zps[j][:])
                    pp = pqp.tile([128, D_OUT], F16, tag="pp", name=f"pp_{blk}_{pi}")
                    vtt(pp[:], zbj[:], zps[i][:], ALU.mult)
                    junk = pqp.tile([128, D_OUT], F16, tag="junk", name=f"junk_{blk}_{pi}")
                    nc.scalar.activation(
                        junk[:], pp[:], ACT_F.Identity,
                        accum_out=s12_2[:, pi : pi + 1],
                    )
                    pp_l.append((pp, junk))

                r4 = pk("r4", 4)
                nc.vector.reciprocal(r4[:], s4[:])
                ri2, rj2 = r4[:, 0:2], r4[:, 2:4]
                rr2 = pk("rr2")
                vtt(rr2[:], ri2, rj2, ALU.mult)
                g2 = pk("g2")
                nc.scalar.activation(g2[:], rr2[:], ACT_F.Sqrt)
                vts(g2[:, 1:2], g2[:, 1:2], -1.0, None, ALU.mult)
                c2_ = pk("c2_")
                vtt(c2_[:], s12_2[:], g2[:], ALU.mult)

                gri2, grj2, cri2, crj2, cg2 = (
                    pk("gri2"), pk("grj2"), pk("cri2"), pk("crj2"), pk("cg2")
                )
                vtt(gri2[:], g2[:], ri2, ALU.mult)
                vtt(grj2[:], g2[:], rj2, ALU.mult)
                vtt(cri2[:], c2_[:], ri2, ALU.mult)
                vtt(crj2[:], c2_[:], rj2, ALU.mult)
                vtt(cg2[:], c2_[:], g2[:], ALU.mult)
                a11_2, a22_2, e12_2 = pk("a11_2"), pk("a22_2"), pk("e12_2")
                m11_2, m22_2, a12_2 = pk("m11_2"), pk("m22_2"), pk("a12_2")
                m12i_2, m12j_2 = pk("m12i_2"), pk("m12j_2")
                mt1, mt2, at = pk("mt1"), pk("mt2"), pk("at")
                vts(a11_2[:], gri2[:], 2.0, None, ALU.mult)
                vtt(mt1[:], cri2[:], ri2, ALU.mult)
                vts(m11_2[:], mt1[:], -3.0, None, ALU.mult)
                vts(a22_2[:], grj2[:], 2.0, None, ALU.mult)
                vtt(mt2[:], crj2[:], rj2, ALU.mult)
                vts(m22_2[:], mt2[:], -3.0, None, ALU.mult)
                vtt(at[:], cg2[:], g2[:], ALU.mult)
                vts(a12_2[:], at[:], 2.0, None, ALU.mult)
                vts(m12i_2[:], gri2[:], -2.0, None, ALU.mult)
                vts(m12j_2[:], grj2[:], -2.0, None, ALU.mult)
                vts(e12_2[:], g2[:], 2.0, None, ALU.mult)

                # --- per-pair D assembly on OUR o-slice (first OSL cols) ---
                d_all = []
                for pi, (i, j) in enumerate([(0, 1), (2, 3)]):
                    q1s = q_all[i][:, 0:OSL]
                    q2s = q_all[j][:, 0:OSL]
                    pps = pp_l[pi][0][:, 0:OSL]
                    sl = slice(pi, pi + 1)
                    d11 = ddp.tile([128, OSL], F16, tag="d11", name=f"d11_{blk}_{pi}")
                    d12 = ddp.tile([128, OSL], F16, tag="d12", name=f"d12_{blk}_{pi}")
                    d22 = ddp.tile([128, OSL], F16, tag="d22", name=f"d22_{blk}_{pi}")
                    t2 = pqp.tile([128, OSL], F16, tag="t2", name=f"t2_{blk}_{pi}")
                    nc.scalar.activation(
                        t2[:], q1s, ACT_F.Identity,
                        bias=cri2[:, sl], scale=m11_2[:, sl],
                    )
                    nc.vector.scalar_tensor_tensor(
                        d11[:], pps, a11_2[:, sl], t2[:], ALU.mult, ALU.add
                    )
                    t4 = pqp.tile([128, OSL], F16, tag="t4", name=f"t4_{blk}_{pi}")
                    nc.scalar.activation(
                        t4[:], q2s, ACT_F.Identity,
                        bias=crj2[:, sl], scale=m22_2[:, sl],
                    )
                    nc.vector.scalar_tensor_tensor(
                        d22[:], pps, a22_2[:, sl], t4[:], ALU.mult, ALU.add
                    )
                    t6 = pqp.tile([128, OSL], F16, tag="t6", name=f"t6_{blk}_{pi}")
                    nc.scalar.activation(
                        t6[:], q1s, ACT_F.Identity,
                        bias=e12_2[:, sl], scale=m12i_2[:, sl],
                    )
                    u1 = pqp.tile([128, OSL], F16, tag="u1", name=f"u1_{blk}_{pi}")
                    nc.vector.scalar_tensor_tensor(
                        u1[:], pps, a12_2[:, sl], t6[:], ALU.mult, ALU.add
                    )
                    nc.vector.scalar_tensor_tensor(
                        d12[:], q2s, m12j_2[:, sl], u1[:], ALU.mult, ALU.add
                    )
                    d_all.append((d11, d12, d22))

                # --- keep-warm transposes over the d-assembly latency ---
                for wi, warm_src in enumerate(
                    (q_all[0][:, 0:128], pp_l[0][1][:, 0:128], pp_l[1][1][:, 0:128])
                ):
                    ptd = ptp.tile([128, 128], F16, tag="pt", name=f"ptd{blk}_{wi}")
                    nc.tensor.transpose(ptd[:], warm_src, ident_sb[:])

                # --- final matmuls accumulate BOTH blocks into one pout ---
                terms = []
                for pi in range(2):
                    for k in range(3):
                        terms.append((d_all[pi][k], xx_all[pi][k]))
                for k, (dmat, xmat) in enumerate(terms):
                    nc.tensor.matmul(
                        pout[:],
                        dmat[:],
                        xmat[:],
                        start=(blk == 0 and k == 0),
                        stop=(blk == NBLK - 1 and k == len(terms) - 1),
                    )

            nc.vector.tensor_copy(osb[:], pout[:])

            # --- ReduceScatter over the 4-rank batch groups (128KB) ---
            rs_in = dramp.tile([OSL, D_IN], F16, tag="rs_in")
            rs_out = dramp.tile([OUT_SH, D_IN], F16, tag="rs_out")
            nc.sync.dma_start(rs_in[:], osb[:])
            nc.gpsimd.collective_compute(
                "ReduceScatter",
                ALU.add,
                replica_groups=[[0, 1, 2, 3], [4, 5, 6, 7]],
                ins=[rs_in[:].opt()],
                outs=[rs_out[:].opt()],
            )
            nc.sync.dma_start(out_d[:], rs_out[:])

    nc.compile()
    return nc


def _get_nc():
    global _CACHED_NC
    if _CACHED_NC is None:
        _CACHED_NC = _build()
    return _CACHED_NC


def _pack_inputs(x, W, ap, p, an, n):
    x = np.ascontiguousarray(np.asarray(x, dtype=np.float32))
    W = np.asarray(W, dtype=np.float32)
    ident = np.eye(128, dtype=np.float16)
    idxs = [np.asarray(a).astype(np.int64) for a in (ap, p, an, n)]
    # per-output-group W with that group's o-rows permuted to the front
    wt_by_group = []
    for g in range(OG):
        perm = np.r_[np.arange(g * OSL, (g + 1) * OSL),
                     np.arange(0, g * OSL),
                     np.arange((g + 1) * OSL, D_OUT)]
        Wp = W[perm, :]
        wt_by_group.append(np.ascontiguousarray(
            Wp.T.reshape(KC, 128, D_OUT).transpose(1, 0, 2)
        ).reshape(128, KC * D_OUT))
    in_maps = []
    for core in range(N_CORES):
        g, q = core // BG, core % BG
        cols = []
        for blk in range(NBLK):
            base = q * (NBLK * 128) + blk * 128
            for a in idxs:
                cols.append(a[base : base + 128])
        idx_core = np.ascontiguousarray(
            np.stack(cols, axis=1).astype(np.int32)
        )  # [128, 4*NBLK]
        in_maps.append(
            {"xfull": x, "wt": wt_by_group[g], "idx": idx_core, "ident": ident}
        )
    return in_maps


def kernel(x, W, ap, p, an, n):
    global LAST_EXEC_NS, LAST_RESULTS
    nc = _get_nc()
    in_maps = _pack_inputs(x, W, ap, p, an, n)
    kw = {}
    if PROFILE:
        kw = dict(trace=True)
    res = run_bass_kernel_spmd(nc, in_maps, list(range(N_CORES)), **kw)
    LAST_EXEC_NS = res.exec_time_ns
    LAST_RESULTS = res
    shards = [
        np.asarray(res.results[i]["out"]).astype(np.float32)
        for i in range(N_CORES)
    ]
    full = np.concatenate(shards, axis=0).reshape(-1)
    return np.ascontiguousarray(full)
